# revision 16
# baseline (speedup 1.0000x reference)
"""Trainium2 Bass kernel for nn_RahmanDynamicNet:
conv(1->20,(34,5)) -> BN(eval) -> sigmoid -> ParametricLIF -> linear(20->1)
-> sigmoid -> ParametricLIF -> [B,T] float32.

Self-contained: takes FULL inputs, shards T across 8 NeuronCores (SPMD, no
collectives), returns FULL [B,T] output.

Why this is exact / fast:
  - Conv output feeds sigmoid => y in (0,1); LIF state v = (1-sw)v + sw*y
    stays < 1 << VTH=1000, so spikes never fire and both LIF layers are pure
    EMAs (linear recurrences) -> DVE tensor_tensor_scan (no T-loop).
  - EMA commutes with the linear readout, so v1 [T,B,H] is never
    materialized: lin(EMA(u)) = EMA(lin(u)).
  - T sharded with a 44-step EMA warmup per core (state error ~0.5^44);
    core 0's scans are reset exactly at t=0 by zeroing the scan decay
    (data0) at the warmup boundary column (per-core d0a/d0b arrays).
  - Conv+BN as one overlap-save matmul per 8-t block: stationary lhsT =
    x-patches [(j=12 taps+halo, ch=34+ones)=420 rows -> 4 chunks of 105,
    b=128], moving rhs = host-built W [105,4,(i=8,h=20)=160] with BN scale
    folded in and BN shift on the ones-channel row. PSUM [b=128,(e,i,h)]
    accumulates 3 blocks per bank; one ACT sigmoid per 480 cols.
  - x is host-transposed/padded to [t, ch, b] float8_e3m4 (halves HBM
    traffic; conv is a 170-term dot -> fp8e3 input noise ~0.1% on output);
    weights bf16; PSUM fp32.
  - h-contraction: gpsimd mul by tiled lin_w, DVE segmented tensor_reduce.
  - Host-side prep (numpy): transpose/pad/quantize x, fold BN+conv bias and
    sw1*lin_w, build patch weight matrix, d0 decay arrays.
"""
import numpy as np
from contextlib import ExitStack
import sys

sys.path.insert(0, "/opt/trn_rl_repo")

import concourse.bass as bass
import concourse.bacc as bacc
import concourse.tile as tile
from concourse import mybir
from concourse.bass_utils import run_bass_kernel_spmd
import ml_dtypes

BF16 = ml_dtypes.bfloat16

B, F, T, H, K = 128, 34, 4000, 20, 5
NCORES = 8
S = 8            # outputs per block
JW = S + 4       # patch window
FA = F + 1       # augmented channels (x + ones)
ROWS = JW * FA   # 420
NCHUNK = 4
CHROWS = ROWS // NCHUNK  # 105
NB = 72          # blocks per core
GE = 3           # blocks per group
NG = NB // GE    # 24 groups
NSB = 3          # DMA segments
SBB = NB // NSB  # 24 blocks per segment
TL = NB * S      # 576
WARM = 44
TO = T // NCORES  # 500
XT_W = TL + 4    # 580
PADL = 48
NCOLS = S * H    # 160
BN_EPS = 1e-5

_DT = mybir.dt


def _sigmoid(v):
    return 1.0 / (1.0 + np.exp(-v))


def _bcast_free(ap, n):
    """[P,1] AP -> [P,n] via step-0 free dim."""
    a = ap[:, 0:1]
    return bass.AP(tensor=a.tensor, offset=a.offset, ap=[list(a.ap[0]), [0, n]])


def build_nc(sw1, sw2, reps=1):
    nc = bacc.Bacc()
    xt = nc.declare_dram_parameter("xt", [XT_W, FA, B], _DT.float8e3, isOutput=False)
    wcp = nc.declare_dram_parameter("wc", [CHROWS, NCHUNK, NCOLS], _DT.bfloat16,
                                    isOutput=False)
    wrp = nc.declare_dram_parameter("wrep", [B, GE * NCOLS], _DT.bfloat16,
                                    isOutput=False)
    cst = nc.declare_dram_parameter("consts", [B, 4], _DT.float32, isOutput=False)
    d0ap = nc.declare_dram_parameter("d0a", [B, TL], _DT.float32, isOutput=False)
    d0bp = nc.declare_dram_parameter("d0b", [B, TL], _DT.float32, isOutput=False)
    outp = nc.declare_dram_parameter("out", [B, TO], _DT.float32, isOutput=True)

    with ExitStack() as ctx:
        tc = ctx.enter_context(tile.TileContext(nc))
        singles = ctx.enter_context(tc.tile_pool(name="singles", bufs=1))
        xp = ctx.enter_context(tc.tile_pool(name="xp", bufs=4))
        pp = ctx.enter_context(tc.tile_pool(name="pp", bufs=4, space="PSUM"))
        up = ctx.enter_context(tc.tile_pool(name="up", bufs=3))
        ump = ctx.enter_context(tc.tile_pool(name="ump", bufs=3))

        wc_sb = singles.tile([CHROWS, NCHUNK, NCOLS], _DT.bfloat16)
        nc.sync.dma_start(out=wc_sb, in_=wcp[:, :, :])
        wrep_sb = singles.tile([B, GE * NCOLS], _DT.bfloat16)
        nc.sync.dma_start(out=wrep_sb, in_=wrp[:, :])
        cst_sb = singles.tile([B, 4], _DT.float32)
        nc.sync.dma_start(out=cst_sb, in_=cst[:, :])
        d0a_sb = singles.tile([B, TL], _DT.float32)
        nc.sync.dma_start(out=d0a_sb, in_=d0ap[:, :])
        d0b_sb = singles.tile([B, TL], _DT.float32)
        nc.sync.dma_start(out=d0b_sb, in_=d0bp[:, :])

        p_sb = singles.tile([B, TL], _DT.float32)
        q_sb = singles.tile([B, TL], _DT.float32)
        z_sb = singles.tile([B, TL], _DT.float32)
        v_sb = singles.tile([B, TL], _DT.float32)
        o_sb = singles.tile([B, TO], _DT.float32)

        # xt strides (elements); layout [t, ch, b] => (j, ch, b) rows are
        # one contiguous run per j
        st_t = FA * B
        xt_ap = xt[:, :, :]

        for _rep in range(reps):
         for seg in range(NSB):
            # one tile holds SBB blocks of lhsT patches; 12 big DMAs/segment
            xb = xp.tile([CHROWS, NCHUNK, SBB, B], _DT.float8e3)
            for cc in range(NCHUNK):
                for jl in range(3):
                    # in: dims (ch, block, b-contiguous)
                    src = bass.AP(
                        tensor=xt_ap.tensor,
                        offset=(S * SBB * seg + 3 * cc + jl) * st_t,
                        ap=[[B, FA], [S * st_t, SBB], [1, B]],
                    )
                    eng = nc.sync if (cc % 2 == 0) else nc.scalar
                    eng.dma_start(out=xb[35 * jl:35 * (jl + 1), cc, :, :],
                                  in_=src)
            for gl in range(SBB // GE):
                g = (SBB // GE) * seg + gl
                ps_g = pp.tile([B, GE, NCOLS], _DT.float32)
                for e in range(GE):
                    ibl = GE * gl + e
                    for cc in range(NCHUNK):
                        nc.tensor.matmul(
                            ps_g[:, e, :], xb[:, cc, ibl, :], wc_sb[:, cc, :],
                            start=(cc == 0), stop=(cc == NCHUNK - 1),
                        )
                u_t = up.tile([B, GE * NCOLS], _DT.bfloat16)
                nc.scalar.activation(
                    out=u_t[:, :], in_=ps_g.rearrange("p e n -> p (e n)"),
                    func=mybir.ActivationFunctionType.Sigmoid,
                )
                um = ump.tile([B, GE * NCOLS], _DT.bfloat16)
                nc.vector.tensor_mul(um[:, :], u_t[:, :], wrep_sb[:, :])
                nc.vector.tensor_reduce(
                    out=p_sb[:, GE * S * g:GE * S * (g + 1)],
                    in_=um.rearrange("p (t h) -> p t h", h=H),
                    axis=mybir.AxisListType.X, op=mybir.AluOpType.add,
                )

         nc.vector.tensor_tensor_scan(
             out=q_sb[:, :], data0=d0a_sb[:, :], data1=p_sb[:, :],
             initial=0.0, op0=mybir.AluOpType.mult, op1=mybir.AluOpType.add,
         )
         nc.scalar.activation(
             out=z_sb[:, :], in_=q_sb[:, :],
             func=mybir.ActivationFunctionType.Sigmoid, bias=cst_sb[:, 2:3],
         )
         nc.vector.tensor_tensor_scan(
             out=v_sb[:, :], data0=d0b_sb[:, :], data1=z_sb[:, :],
             initial=0.0, op0=mybir.AluOpType.mult, op1=mybir.AluOpType.add,
         )
         nc.scalar.activation(
             out=o_sb[:, :], in_=v_sb[:, WARM:WARM + TO],
             func=mybir.ActivationFunctionType.Copy, scale=float(sw2),
         )
         nc.sync.dma_start(out=outp[:, :], in_=o_sb[:, :])
    nc.compile()
    return nc


def prep(x, conv_w, conv_b, bn_gamma, bn_beta, bn_mean, bn_var,
         lin_w, lin_b, w1, w2):
    x = np.asarray(x, np.float32)
    inv = (np.asarray(bn_gamma, np.float32)
           / np.sqrt(np.asarray(bn_var, np.float32) + BN_EPS))
    shift = (np.asarray(conv_b, np.float32)
             - np.asarray(bn_mean, np.float32)) * inv \
        + np.asarray(bn_beta, np.float32)
    sw1 = float(_sigmoid(np.float32(np.asarray(w1))))
    sw2 = float(_sigmoid(np.float32(np.asarray(w2))))
    linb = float(np.asarray(lin_b, np.float32).reshape(-1)[0])
    lw = np.asarray(lin_w, np.float32).reshape(-1)

    GT = PADL + T + 52
    x_aug = np.zeros((GT, FA, B), np.float32)
    x_aug[PADL:PADL + T, :F, :] = x[:, 0].transpose(2, 1, 0)
    x_aug[PADL:PADL + T, F, :] = 1.0
    x_aug_bf = x_aug.astype(ml_dtypes.float8_e3m4)

    cw = np.asarray(conv_w, np.float32)[:, 0]  # [H,F,K]
    Wf = np.zeros((ROWS, NCOLS), np.float32)
    for i in range(S):
        for k in range(K):
            j = i + k
            Wf[j * FA:j * FA + F, i * H:(i + 1) * H] = \
                (cw[:, :, k] * inv[:, None]).T
        Wf[(i + 2) * FA + F, i * H:(i + 1) * H] = shift
    wc = np.ascontiguousarray(
        Wf.reshape(NCHUNK, CHROWS, NCOLS).transpose(1, 0, 2)).astype(BF16)

    wr = np.tile(lw * sw1, GE * S).astype(BF16)
    wrep = np.ascontiguousarray(np.broadcast_to(wr, (B, GE * NCOLS)))

    consts = np.zeros((B, 4), np.float32)
    consts[:, 0] = 1.0 - sw1
    consts[:, 1] = 1.0 - sw2
    consts[:, 2] = linb

    d0a = np.full((B, TL), 1.0 - sw1, np.float32)
    d0b = np.full((B, TL), 1.0 - sw2, np.float32)
    d0a0 = d0a.copy(); d0a0[:, WARM] = 0.0
    d0b0 = d0b.copy(); d0b0[:, WARM] = 0.0

    in_maps = []
    for c in range(NCORES):
        g0 = 500 * c + 2
        xt = np.ascontiguousarray(x_aug_bf[g0:g0 + XT_W, :, :])
        in_maps.append({"xt": xt, "wc": wc, "wrep": wrep, "consts": consts,
                        "d0a": d0a0 if c == 0 else d0a,
                        "d0b": d0b0 if c == 0 else d0b})
    return in_maps, sw1, sw2


_NC_CACHE = {}


def kernel(**inputs):
    in_maps, sw1, sw2 = prep(**inputs)
    key = (round(sw1, 9), round(sw2, 9))
    if key not in _NC_CACHE:
        _NC_CACHE[key] = build_nc(sw1, sw2)
    nc = _NC_CACHE[key]
    res = run_bass_kernel_spmd(nc, in_maps, list(range(NCORES)))
    outs = [np.asarray(res.results[c]["out"], np.float32)
            for c in range(NCORES)]
    return np.concatenate(outs, axis=1)


# revision 17
# speedup vs baseline: 2.4956x; 2.4956x over previous
"""Trainium2 Bass kernel for nn_RahmanDynamicNet:
conv(1->20,(34,5)) -> BN(eval) -> sigmoid -> ParametricLIF -> linear(20->1)
-> sigmoid -> ParametricLIF -> [B,T] float32.

Self-contained: takes FULL inputs, shards T across 8 NeuronCores (SPMD, no
collectives), returns FULL [B,T] output.

Why this is exact / fast:
  - Conv output feeds sigmoid => y in (0,1); LIF state v = (1-sw)v + sw*y
    stays < 1 << VTH=1000, so spikes never fire and both LIF layers are pure
    EMAs (linear recurrences) -> DVE tensor_tensor_scan (no T-loop).
  - EMA commutes with the linear readout, so v1 [T,B,H] is never
    materialized: lin(EMA(u)) = EMA(lin(u)).
  - T sharded with a 44-step EMA warmup per core (state error ~0.5^44);
    core 0's scans are reset exactly at t=0 by zeroing the scan decay
    (data0) at the warmup boundary column (per-core d0a/d0b arrays).
  - Conv+BN as one overlap-save matmul per 8-t block: stationary lhsT =
    x-patches [(j=12 taps+halo, ch=34+ones)=420 rows -> 4 chunks of 105,
    b=128], moving rhs = host-built W [105,4,(i=8,h=20)=160] with BN scale
    folded in and BN shift on the ones-channel row. PSUM [b=128,(e,i,h)]
    accumulates 3 blocks per bank; one ACT sigmoid per 480 cols.
  - x is host-transposed/padded to [t, ch, b] float8_e3m4 (halves HBM
    traffic; conv is a 170-term dot -> fp8e3 input noise ~0.1% on output);
    weights bf16; PSUM fp32.
  - h-contraction: gpsimd mul by tiled lin_w, DVE segmented tensor_reduce.
  - Host-side prep (numpy): transpose/pad/quantize x, fold BN+conv bias and
    sw1*lin_w, build patch weight matrix, d0 decay arrays.
"""
import numpy as np
from contextlib import ExitStack
import sys

sys.path.insert(0, "/opt/trn_rl_repo")

import concourse.bass as bass
import concourse.bacc as bacc
import concourse.tile as tile
from concourse import mybir
from concourse.bass_utils import run_bass_kernel_spmd
import ml_dtypes

BF16 = ml_dtypes.bfloat16

B, F, T, H, K = 128, 34, 4000, 20, 5
NCORES = 8
S = 8            # outputs per block
JW = S + 4       # patch window
FA = F + 1       # augmented channels (x + ones)
ROWS = JW * FA   # 420
NCHUNK = 4
CHROWS = ROWS // NCHUNK  # 105
NB = 72          # blocks per core
GE = 3           # blocks per group
NG = NB // GE    # 24 groups
NSB = 3          # DMA segments
SBB = NB // NSB  # 24 blocks per segment
TL = NB * S      # 576
WARM = 44
TO = T // NCORES  # 500
XT_W = TL + 4    # 580
PADL = 48
NCOLS = S * H    # 160
BN_EPS = 1e-5

_DT = mybir.dt


def _sigmoid(v):
    return 1.0 / (1.0 + np.exp(-v))


def _bcast_free(ap, n):
    """[P,1] AP -> [P,n] via step-0 free dim."""
    a = ap[:, 0:1]
    return bass.AP(tensor=a.tensor, offset=a.offset, ap=[list(a.ap[0]), [0, n]])


def build_nc(sw1, sw2, reps=1):
    nc = bacc.Bacc()
    xt = nc.declare_dram_parameter("xt", [XT_W, FA, B], _DT.float8e3, isOutput=False)
    wcp = nc.declare_dram_parameter("wc", [CHROWS, NCHUNK, NCOLS], _DT.bfloat16,
                                    isOutput=False)
    wrp = nc.declare_dram_parameter("wrep", [B, GE * NCOLS], _DT.bfloat16,
                                    isOutput=False)
    cst = nc.declare_dram_parameter("consts", [B, 4], _DT.float32, isOutput=False)
    d0ap = nc.declare_dram_parameter("d0a", [B, TL], _DT.float32, isOutput=False)
    d0bp = nc.declare_dram_parameter("d0b", [B, TL], _DT.float32, isOutput=False)
    outp = nc.declare_dram_parameter("out", [B, TO], _DT.float32, isOutput=True)

    with ExitStack() as ctx:
        tc = ctx.enter_context(tile.TileContext(nc))
        singles = ctx.enter_context(tc.tile_pool(name="singles", bufs=1))
        xp = ctx.enter_context(tc.tile_pool(name="xp", bufs=4))
        pp = ctx.enter_context(tc.tile_pool(name="pp", bufs=4, space="PSUM"))
        up = ctx.enter_context(tc.tile_pool(name="up", bufs=3))
        ump = ctx.enter_context(tc.tile_pool(name="ump", bufs=3))

        wc_sb = singles.tile([CHROWS, NCHUNK, NCOLS], _DT.bfloat16)
        nc.sync.dma_start(out=wc_sb, in_=wcp[:, :, :])
        wrep_sb = singles.tile([B, GE * NCOLS], _DT.bfloat16)
        nc.sync.dma_start(out=wrep_sb, in_=wrp[:, :])
        cst_sb = singles.tile([B, 4], _DT.float32)
        nc.sync.dma_start(out=cst_sb, in_=cst[:, :])
        d0a_sb = singles.tile([B, TL], _DT.float32)
        nc.sync.dma_start(out=d0a_sb, in_=d0ap[:, :])
        d0b_sb = singles.tile([B, TL], _DT.float32)
        nc.sync.dma_start(out=d0b_sb, in_=d0bp[:, :])

        p_sb = singles.tile([B, TL], _DT.float32)
        q_sb = singles.tile([B, TL], _DT.float32)
        z_sb = singles.tile([B, TL], _DT.float32)
        v_sb = singles.tile([B, TL], _DT.float32)
        o_sb = singles.tile([B, TO], _DT.float32)

        # xt strides (elements); layout [t, ch, b] => (j, ch, b) rows are
        # one contiguous run per j
        st_t = FA * B
        xt_ap = xt[:, :, :]

        for _rep in range(reps):
         for seg in range(NSB):
            # one tile holds SBB blocks of lhsT patches; 12 big DMAs/segment
            xb = xp.tile([CHROWS, NCHUNK, SBB, B], _DT.float8e3)
            for cc in range(NCHUNK):
                for jl in range(3):
                    # in: dims (ch, block, b-contiguous)
                    src = bass.AP(
                        tensor=xt_ap.tensor,
                        offset=(S * SBB * seg + 3 * cc + jl) * st_t,
                        ap=[[B, FA], [S * st_t, SBB], [1, B]],
                    )
                    eng = nc.sync if (cc % 2 == 0) else nc.scalar
                    eng.dma_start(out=xb[35 * jl:35 * (jl + 1), cc, :, :],
                                  in_=src)
            for gl in range(SBB // GE):
                g = (SBB // GE) * seg + gl
                ps_g = pp.tile([B, GE, NCOLS], _DT.float32)
                for e in range(GE):
                    ibl = GE * gl + e
                    for cc in range(NCHUNK):
                        nc.tensor.matmul(
                            ps_g[:, e, :], xb[:, cc, ibl, :], wc_sb[:, cc, :],
                            start=(cc == 0), stop=(cc == NCHUNK - 1),
                        )
                u_t = up.tile([B, GE * NCOLS], _DT.bfloat16)
                nc.scalar.activation(
                    out=u_t[:, :], in_=ps_g.rearrange("p e n -> p (e n)"),
                    func=mybir.ActivationFunctionType.Sigmoid,
                )
                um = ump.tile([B, GE * NCOLS], _DT.bfloat16)
                nc.vector.tensor_mul(um[:, :], u_t[:, :], wrep_sb[:, :])
                nc.vector.tensor_reduce(
                    out=p_sb[:, GE * S * g:GE * S * (g + 1)],
                    in_=um.rearrange("p (t h) -> p t h", h=H),
                    axis=mybir.AxisListType.X, op=mybir.AluOpType.add,
                )

         # segment-chained scans + output: overlap the EMA/sigmoid/output
         # tail with later segments' conv compute
         SEGC = TL // NSB
         for seg in range(NSB):
             s0, s1 = SEGC * seg, SEGC * (seg + 1)
             nc.vector.tensor_tensor_scan(
                 out=q_sb[:, s0:s1], data0=d0a_sb[:, s0:s1],
                 data1=p_sb[:, s0:s1],
                 initial=(0.0 if seg == 0 else q_sb[:, s0 - 1:s0]),
                 op0=mybir.AluOpType.mult, op1=mybir.AluOpType.add,
             )
             nc.scalar.activation(
                 out=z_sb[:, s0:s1], in_=q_sb[:, s0:s1],
                 func=mybir.ActivationFunctionType.Sigmoid,
                 bias=cst_sb[:, 2:3],
             )
             nc.vector.tensor_tensor_scan(
                 out=v_sb[:, s0:s1], data0=d0b_sb[:, s0:s1],
                 data1=z_sb[:, s0:s1],
                 initial=(0.0 if seg == 0 else v_sb[:, s0 - 1:s0]),
                 op0=mybir.AluOpType.mult, op1=mybir.AluOpType.add,
             )
             c0 = max(0, s0 - WARM)
             c1 = min(TO, s1 - WARM)
             nc.scalar.activation(
                 out=o_sb[:, c0:c1], in_=v_sb[:, WARM + c0:WARM + c1],
                 func=mybir.ActivationFunctionType.Copy, scale=float(sw2),
             )
             nc.sync.dma_start(out=outp[:, c0:c1], in_=o_sb[:, c0:c1])
    nc.compile()
    return nc


def prep(x, conv_w, conv_b, bn_gamma, bn_beta, bn_mean, bn_var,
         lin_w, lin_b, w1, w2):
    x = np.asarray(x, np.float32)
    inv = (np.asarray(bn_gamma, np.float32)
           / np.sqrt(np.asarray(bn_var, np.float32) + BN_EPS))
    shift = (np.asarray(conv_b, np.float32)
             - np.asarray(bn_mean, np.float32)) * inv \
        + np.asarray(bn_beta, np.float32)
    sw1 = float(_sigmoid(np.float32(np.asarray(w1))))
    sw2 = float(_sigmoid(np.float32(np.asarray(w2))))
    linb = float(np.asarray(lin_b, np.float32).reshape(-1)[0])
    lw = np.asarray(lin_w, np.float32).reshape(-1)

    GT = PADL + T + 52
    x_aug = np.zeros((GT, FA, B), np.float32)
    x_aug[PADL:PADL + T, :F, :] = x[:, 0].transpose(2, 1, 0)
    x_aug[PADL:PADL + T, F, :] = 1.0
    x_aug_bf = x_aug.astype(ml_dtypes.float8_e3m4)

    cw = np.asarray(conv_w, np.float32)[:, 0]  # [H,F,K]
    Wf = np.zeros((ROWS, NCOLS), np.float32)
    for i in range(S):
        for k in range(K):
            j = i + k
            Wf[j * FA:j * FA + F, i * H:(i + 1) * H] = \
                (cw[:, :, k] * inv[:, None]).T
        Wf[(i + 2) * FA + F, i * H:(i + 1) * H] = shift
    wc = np.ascontiguousarray(
        Wf.reshape(NCHUNK, CHROWS, NCOLS).transpose(1, 0, 2)).astype(BF16)

    wr = np.tile(lw * sw1, GE * S).astype(BF16)
    wrep = np.ascontiguousarray(np.broadcast_to(wr, (B, GE * NCOLS)))

    consts = np.zeros((B, 4), np.float32)
    consts[:, 0] = 1.0 - sw1
    consts[:, 1] = 1.0 - sw2
    consts[:, 2] = linb

    d0a = np.full((B, TL), 1.0 - sw1, np.float32)
    d0b = np.full((B, TL), 1.0 - sw2, np.float32)
    d0a0 = d0a.copy(); d0a0[:, WARM] = 0.0
    d0b0 = d0b.copy(); d0b0[:, WARM] = 0.0

    in_maps = []
    for c in range(NCORES):
        g0 = 500 * c + 2
        xt = np.ascontiguousarray(x_aug_bf[g0:g0 + XT_W, :, :])
        in_maps.append({"xt": xt, "wc": wc, "wrep": wrep, "consts": consts,
                        "d0a": d0a0 if c == 0 else d0a,
                        "d0b": d0b0 if c == 0 else d0b})
    return in_maps, sw1, sw2


_NC_CACHE = {}


def kernel(**inputs):
    in_maps, sw1, sw2 = prep(**inputs)
    key = (round(sw1, 9), round(sw2, 9))
    if key not in _NC_CACHE:
        _NC_CACHE[key] = build_nc(sw1, sw2)
    nc = _NC_CACHE[key]
    res = run_bass_kernel_spmd(nc, in_maps, list(range(NCORES)))
    outs = [np.asarray(res.results[c]["out"], np.float32)
            for c in range(NCORES)]
    return np.concatenate(outs, axis=1)
